# revision 1
# baseline (speedup 1.0000x reference)
"""Trainium2 Bass kernel for nn_Attention (conv-qkv spatial attention block).

Contract: kernel(**inputs) takes FULL unsharded inputs (B=16, C=512, H=W=64),
shards batch across 8 NeuronCores (2 images per core), runs one SPMD Bass
program, and returns the FULL output (fp32).

Math per image (reference):
  q  = conv3x3(x, q_w) + q_b                      # (C, H, W)
  kv = conv3x3(x, kv_w) + kv_b ; k, v = split(kv)
  per channel ch: attn = softmax(q_ch @ k_ch^T) ; o_ch = attn @ v_ch
  y  = conv1x1(perm(o), proj_w) + proj_b          # head/channel permutation
       (the permutation is folded into proj_w on the host)

Device implementation notes:
  - All matmuls in bf16 with fp32 PSUM accumulation.
  - 3x3 conv = 9 shifted matmuls over a zero-padded bf16 copy of x in SBUF,
    accumulated over ci chunks; bias is added during the ACT evacuation.
  - Per-channel attention operands are produced by DVE stream-transpose
    (32x32 blocks), giving a tiled layout where the spatial index lives on
    partitions mod 32 and attention runs as K=32 matmuls packed 4-wide on
    the PE array via tile_position quadrants.
  - softmax: exp in fp32 without max subtraction (logits bounded ~|75| < 88),
    row sums via a ones-matmul, one reciprocal + broadcast multiply.
"""

import numpy as np
import ml_dtypes

import concourse.bass as bass
import concourse.bacc as bacc
import concourse.mybir as mybir
import concourse.tile as tile
from concourse.bass_utils import run_bass_kernel_spmd

F32 = mybir.dt.float32
BF16 = mybir.dt.bfloat16
AF = mybir.ActivationFunctionType
BF = ml_dtypes.bfloat16

H = 64          # spatial height (attention over rows, contracting cols)
PW = 66         # padded row width
NPOS = H * H    # 4096 positions per image


def build_nc(B=2, C=512, n_cores=8, repeat=1, phases=("conv", "attn", "proj")):
    """Build the per-core Bass program. B = images per core.

    repeat > 1 emits the whole body multiple times (timing builds only).
    phases: drop "attn"/"proj" for timing-breakdown builds.
    """
    M = C // 128            # channel chunks (co chunks and ci chunks)
    nc = bacc.Bacc("TRN2", target_bir_lowering=False, debug=False,
                   num_devices=n_cores)

    x_d = nc.dram_tensor("x", [B, C, H, H], BF16, kind="ExternalInput")
    wq_d = nc.dram_tensor("wq", [M, 128, 9 * M, 128], BF16, kind="ExternalInput")
    wk_d = nc.dram_tensor("wk", [M, 128, 9 * M, 128], BF16, kind="ExternalInput")
    wv_d = nc.dram_tensor("wv", [M, 128, 9 * M, 128], BF16, kind="ExternalInput")
    pw_d = nc.dram_tensor("pw", [M, 128, C], BF16, kind="ExternalInput")
    bias_d = nc.dram_tensor("biases", [4, C], F32, kind="ExternalInput")
    y_d = nc.dram_tensor("y", [B, C, H, H], F32, kind="ExternalOutput")

    with tile.TileContext(nc) as tc:
        _body(tc, nc, B, M, x_d, (wq_d, wk_d, wv_d), pw_d, bias_d, y_d,
              repeat=repeat, phases=phases)
    nc.compile()
    return nc


def _body(tc, nc, B, M, x_d, w_ds, pw_d, bias_d, y_d, repeat=1,
          phases=("conv", "attn", "proj")):
    from contextlib import ExitStack
    ctx = ExitStack()
    C = M * 128
    const = ctx.enter_context(tc.tile_pool(name="const", bufs=1))
    xpad_p = ctx.enter_context(tc.tile_pool(name="xpad", bufs=min(4 * B, 5)))
    w_p = ctx.enter_context(tc.tile_pool(name="wconv", bufs=3))
    qkv_p = ctx.enter_context(tc.tile_pool(name="qkv", bufs=7))
    exp_p = ctx.enter_context(tc.tile_pool(name="exp", bufs=3))
    nt_p = ctx.enter_context(tc.tile_pool(name="normT", bufs=2))
    rc_p = ctx.enter_context(tc.tile_pool(name="recip", bufs=2))
    as_p = ctx.enter_context(tc.tile_pool(name="attns", bufs=2))
    acm_p = ctx.enter_context(tc.tile_pool(name="attncm", bufs=min(M + 1, 5)))
    y_p = ctx.enter_context(tc.tile_pool(name="yout", bufs=4))
    st_p = ctx.enter_context(tc.tile_pool(name="stage", bufs=6))
    cp_ps = ctx.enter_context(tc.tile_pool(name="cpps", bufs=2, space="PSUM"))
    at_ps = ctx.enter_context(tc.tile_pool(name="atps", bufs=3, space="PSUM"))
    nm_ps = ctx.enter_context(tc.tile_pool(name="nmps", bufs=1, space="PSUM"))
    o2_ps = ctx.enter_context(tc.tile_pool(name="o2ps", bufs=2, space="PSUM"))

    # constants
    ones32 = const.tile([128, 32], BF16, tag="ones32")
    nc.gpsimd.memset(ones32[:, :], 1.0)
    # per-partition bias columns: col (ti*M + m) = bias[ti, m*128:(m+1)*128]
    bias_sb = const.tile([128, 4 * M], F32, tag="bias")
    nc.sync.dma_start(out=bias_sb[:, :],
                      in_=bias_d[:, :].rearrange("a (m p) -> p (a m)", p=128))
    pw_sb = const.tile([128, M * C], BF16, tag="pw")
    for k4 in range(M):
        nc.sync.dma_start(out=pw_sb[:, k4 * C:(k4 + 1) * C], in_=pw_d[k4, :, :])

    for b in [b for _ in range(repeat) for b in range(B)]:
        # ---- load x image b: zero-pad borders, DMA-cast interior to bf16
        xpad = []
        for k4 in range(M):
            xp = xpad_p.tile([128, PW * PW], BF16, tag="xpad")
            z = xp[:, :].rearrange("p (r c) -> p r c", c=PW)
            nc.gpsimd.memset(z[:, 0, :], 0.0)
            nc.gpsimd.memset(z[:, PW - 1, :], 0.0)
            nc.gpsimd.memset(z[:, :, 0], 0.0)
            nc.gpsimd.memset(z[:, :, PW - 1], 0.0)
            nc.sync.dma_start(out=z[:, 1:H + 1, 1:H + 1],
                              in_=x_d[b, k4 * 128:(k4 + 1) * 128, :, :])
            xpad.append(xp)

        acm = []  # attnout channel-major chunks for proj
        for m in range(M):
            # ---- convs for channel chunk m -> transposed layouts
            outs = {}
            for ti, tname in enumerate(("q", "k", "v")):
                w_tile = w_p.tile([128, 9 * M * 128], BF16, tag="wconv")
                nc.sync.dma_start(out=w_tile[:, :],
                                  in_=w_ds[ti][m].rearrange("p a b -> p (a b)"))
                o_t = qkv_p.tile([128, NPOS], BF16, tag="qkv")
                colmajor = (tname == "v")
                for n in range(8):
                    psum = cp_ps.tile([128, 512], F32, tag="cpps")
                    for k4 in range(M):
                        zz = xpad[k4][:, :].rearrange("p (r c) -> p r c", c=PW)
                        for t in range(9):
                            dy, dx = t // 3, t % 3
                            if colmajor:
                                rhs = zz[:, dy:dy + H, n * 8 + dx:n * 8 + dx + 8]
                                rhs = rhs.transpose([0, 2, 1])
                            else:
                                rhs = zz[:, n * 8 + dy:n * 8 + dy + 8, dx:dx + H]
                            nc.tensor.matmul(
                                psum[:, :],
                                w_tile[:, (t * M + k4) * 128:(t * M + k4 + 1) * 128],
                                rhs, start=(k4 == 0 and t == 0),
                                stop=(k4 == M - 1 and t == 8))
                    # evacuate: bias add + cast to bf16 (DVE), then DVE
                    # 32x32 stream transpose (bf16 -> bf16)
                    stage = st_p.tile([128, 512], BF16, tag="stage")
                    nc.vector.tensor_scalar_add(
                        stage[:, :], psum[:, :],
                        bias_sb[:, ti * M + m: ti * M + m + 1])
                    nc.vector.transpose(o_t[:, n * 512:(n + 1) * 512], stage[:, :])
                outs[tname] = o_t

            if "attn" not in phases:
                nc.gpsimd.dma_start(
                    out=y_d[b, m * 128:(m + 1) * 128].rearrange("p a b -> p (a b)"),
                    in_=outs["q"][:, :])
                continue

            # ---- attention for the 128 channels of chunk m
            o_q, o_k, o_v = outs["q"], outs["k"], outs["v"]
            a_s = as_p.tile([128, NPOS], BF16, tag="attns")
            kks = [o_k[cb * 32:(cb + 1) * 32, :].rearrange(
                "p (kid half c) -> p kid half c", half=2, c=32) for cb in range(4)]
            qqs = [o_q[cb * 32:(cb + 1) * 32, :].rearrange(
                "p (i half c) -> p i half c", half=2, c=32) for cb in range(4)]
            vvs = [o_v[cb * 32:(cb + 1) * 32, :].rearrange(
                "p (w half c) -> p w half c", half=2, c=32) for cb in range(4)]
            for qd in range(8):
                atp = at_ps.tile([128, 512], F32, tag="atps")
                # logits^T:  atp[cb*32+kappa, sl*128+kb*64+i] = sum_j k*q
                # cb innermost so consecutive MMs hit different PE quadrants
                for i1, (sl, kb, jb) in enumerate(
                        (s, k, j) for s in range(4) for k in range(2) for j in range(2)):
                    c = qd * 4 + sl
                    for cb in range(4):
                        nc.tensor.matmul(
                            atp[cb * 32:(cb + 1) * 32,
                                sl * 128 + kb * 64: sl * 128 + (kb + 1) * 64],
                            kks[cb][:, kb * 32:(kb + 1) * 32, jb, c],
                            qqs[cb][:, :, jb, c],
                            start=(i1 == 0), stop=(i1 == 15),
                            skip_group_check=True,
                            tile_position=(cb * 32, cb * 32))
                # exp (fp32 -> bf16), no max subtraction
                ex = exp_p.tile([128, 512], BF16, tag="exp")
                nc.scalar.activation(ex[:, :], atp[:, :], AF.Exp)
                # row sums (over kidx) via ones-matmul, replicated on 32 parts
                nmp = nm_ps.tile([128, 256], F32, tag="nmps")
                for kb in range(2):
                    for cb in range(4):
                        ee = ex[cb * 32:(cb + 1) * 32, :].rearrange(
                            "p (sl half i) -> p sl half i", half=2, i=64)
                        nc.tensor.matmul(
                            nmp[cb * 32:(cb + 1) * 32, :],
                            ones32[cb * 32:(cb + 1) * 32, :],
                            ee[:, :, kb, :],
                            start=(kb == 0), stop=(kb == 1),
                            skip_group_check=True,
                            tile_position=(cb * 32, cb * 32))
                nt = nt_p.tile([128, 256], F32, tag="normT")
                nc.vector.transpose(nt[:, :], nmp[:, :])
                rc = rc_p.tile([128, 8], F32, tag="recip")
                nc.vector.reciprocal(
                    rc[:, :], nt[:, :].rearrange("p (t c) -> p t c", c=32)[:, :, 0])
                # out2 = attn_exp^T' @ v   (unnormalized), K=32 chunks
                o2p = o2_ps.tile([128, 512], F32, tag="o2ps")
                for i2, (sl, ib, kb) in enumerate(
                        (s, i, k) for s in range(4) for i in range(2) for k in range(2)):
                    c = qd * 4 + sl
                    for cb in range(4):
                        nc.tensor.matmul(
                            o2p[cb * 32:(cb + 1) * 32,
                                sl * 128 + ib * 64: sl * 128 + (ib + 1) * 64],
                            ex[cb * 32:(cb + 1) * 32,
                               sl * 128 + kb * 64 + ib * 32:
                               sl * 128 + kb * 64 + ib * 32 + 32],
                            vvs[cb][:, :, kb, c],
                            start=(i2 == 0), stop=(i2 == 15),
                            skip_group_check=True,
                            tile_position=(cb * 32, cb * 32))
                # normalize + write into attnout_s (v-style layout), bf16
                in0 = o2p[:, :].rearrange("p (sl ib w) -> p sl ib w", ib=2, w=64)
                in1 = rc[:, :].rearrange("p (sl ib) -> p sl ib", ib=2)
                in1 = in1.unsqueeze(3).broadcast_to((128, 4, 2, 64))
                outap = a_s[:, :].rearrange("p (t c) -> p t c", c=32)
                outap = outap[:, :, qd * 4:qd * 4 + 4].rearrange(
                    "p (w ib) sl -> p w ib sl", ib=2).transpose([0, 3, 2, 1])
                nc.vector.tensor_tensor(outap, in0, in1, mybir.AluOpType.mult)
            # back-transpose to channel-major (column-major positions)
            a_cm = acm_p.tile([128, NPOS], BF16, tag="attncm")
            nc.vector.transpose(a_cm[:, :], a_s[:, :])
            acm.append(a_cm)

        if "attn" not in phases:
            continue
        if "proj" not in phases:
            for m in range(M):
                nc.gpsimd.dma_start(
                    out=y_d[b, m * 128:(m + 1) * 128].rearrange("p a b -> p (a b)"),
                    in_=acm[m][:, :])
            del acm
            continue

        # ---- proj (1x1 conv with permuted weights) + bias, row-major out
        for mo in range(M):
            for n in range(8):
                yt = y_p.tile([128, NPOS // 8], F32, tag="yout")
                psum = cp_ps.tile([128, 512], F32, tag="cpps")
                for k4 in range(M):
                    rhs = acm[k4][:, :].rearrange("p (w i) -> p w i", i=64)
                    rhs = rhs[:, :, n * 8:(n + 1) * 8].transpose([0, 2, 1])
                    nc.tensor.matmul(
                        psum[:, :],
                        pw_sb[:, k4 * C + mo * 128: k4 * C + (mo + 1) * 128],
                        rhs, start=(k4 == 0), stop=(k4 == M - 1))
                nc.scalar.activation(yt[:, :], psum[:, :], AF.Identity,
                                     bias=bias_sb[:, 3 * M + mo: 3 * M + mo + 1])
                nc.sync.dma_start(
                    out=y_d[b, mo * 128:(mo + 1) * 128, n * 8:(n + 1) * 8, :],
                    in_=yt[:, :])
        del acm
    ctx.close()


def prep_weights(q_w, q_b, kv_w, kv_b, proj_w, proj_b, C=512):
    """Host-side weight re-layouts (numpy, bf16)."""
    M = C // 128
    nh = 16
    cpg = C // nh

    def conv_w(w):
        # w[co, ci, dy, dx] -> [m, p(ci%128), t(=dy*3+dx), k4, co] flat
        w4 = w.reshape(M, 128, M, 128, 3, 3)          # [m, co, k4, p, dy, dx]
        out = np.transpose(w4, (0, 3, 4, 5, 2, 1))    # [m, p, dy, dx, k4, co]
        out = out.reshape(M, 128, 9 * M, 128)
        return np.ascontiguousarray(out).astype(BF)

    wq = conv_w(q_w)
    wk = conv_w(kv_w[:C])
    wv = conv_w(kv_w[C:])
    ch = np.arange(C)
    perm = (ch % cpg) * nh + ch // cpg                # proj input index per attn channel
    pwp = proj_w[:, :, 0, 0][:, perm]                 # [co, ch]
    pw = np.ascontiguousarray(pwp.T.reshape(M, 128, C)).astype(BF)
    biases = np.stack([q_b, kv_b[:C], kv_b[C:], proj_b]).astype(np.float32)
    return wq, wk, wv, pw, biases


_CACHE = {}


def _get_nc():
    if "nc" not in _CACHE:
        _CACHE["nc"] = build_nc(B=2, C=512, n_cores=8)
    return _CACHE["nc"]


def make_in_maps(x, q_w, q_b, kv_w, kv_b, proj_w, proj_b, n_cores=8):
    wq, wk, wv, pw, biases = prep_weights(
        np.asarray(q_w), np.asarray(q_b), np.asarray(kv_w), np.asarray(kv_b),
        np.asarray(proj_w), np.asarray(proj_b))
    x = np.asarray(x, dtype=np.float32).astype(BF)
    bpc = x.shape[0] // n_cores
    return [
        {"x": np.ascontiguousarray(x[i * bpc:(i + 1) * bpc]),
         "wq": wq, "wk": wk, "wv": wv, "pw": pw, "biases": biases}
        for i in range(n_cores)
    ]


def kernel(x, q_w, q_b, kv_w, kv_b, proj_w, proj_b):
    nc = _get_nc()
    in_maps = make_in_maps(x, q_w, q_b, kv_w, kv_b, proj_w, proj_b)
    res = run_bass_kernel_spmd(nc, in_maps, core_ids=list(range(8)))
    out = np.concatenate([res.results[i]["y"] for i in range(8)], axis=0)
    return out.astype(np.float32)



# revision 9
# speedup vs baseline: 556.3029x; 556.3029x over previous
"""Trainium2 Bass kernel for nn_Attention (conv-qkv spatial attention block).

Contract: kernel(**inputs) takes FULL unsharded inputs (B=16, C=512, H=W=64),
shards batch across 8 NeuronCores (2 images per core), runs one SPMD Bass
program, and returns the FULL output (fp32).

Math per image (reference):
  q  = conv3x3(x, q_w) + q_b                      # (C, H, W)
  kv = conv3x3(x, kv_w) + kv_b ; k, v = split(kv)
  per channel ch: attn = softmax(q_ch @ k_ch^T) ; o_ch = attn @ v_ch
  y  = conv1x1(perm(o), proj_w) + proj_b          # head/channel permutation
       (the permutation is folded into proj_w on the host)

Device implementation notes:
  - All matmuls in bf16 with fp32 PSUM accumulation.
  - 3x3 conv = 9 shifted matmuls over a zero-padded bf16 copy of x in SBUF,
    accumulated over ci chunks; bias is added during the ACT evacuation.
  - Per-channel attention operands are produced by DVE stream-transpose
    (32x32 blocks), giving a tiled layout where the spatial index lives on
    partitions mod 32 and attention runs as K=32 matmuls packed 4-wide on
    the PE array via tile_position quadrants.
  - softmax: exp in fp32 without max subtraction (logits bounded ~|75| < 88),
    row sums via a ones-matmul, one reciprocal + broadcast multiply.
"""

import numpy as np
import ml_dtypes

import concourse.bass as bass
import concourse.bacc as bacc
import concourse.mybir as mybir
import concourse.tile as tile
from concourse.bass_utils import run_bass_kernel_spmd

F32 = mybir.dt.float32
BF16 = mybir.dt.bfloat16
AF = mybir.ActivationFunctionType
BF = ml_dtypes.bfloat16

H = 64          # spatial height (attention over rows, contracting cols)
PW = 66         # padded row width
NPOS = H * H    # 4096 positions per image
NSTRIP = PW * PW        # 4356 padded strip positions
XCOLS = 4800            # xpad allocation (group-8 rhs overruns past 4356)
SCOLS = 4608            # 9 psum groups x 512 strip outputs
GSET = 3                # psum groups sharing one weight residency


def _strip_redundant_ldws(nc):
    """Drop InstLdweights identical to the immediately-preceding kept
    Ldweights (same weights AP/tile_position, no waits/updates, no intervening
    PE drain). The following Matmult reuses the already-loaded weights —
    numerically identical either way, but skips the redundant weight-load
    stream on the PE."""
    n_drop = 0
    for bb in nc.m.functions[0].blocks:
        il = bb.instructions
        last_key = None
        drop_idx = []
        for idx, inst in enumerate(il):
            tn = type(inst).__name__
            if tn == "InstLdweights":
                key = inst.concise()
                if key == last_key and not inst.has_wait() and not inst.has_update():
                    drop_idx.append(idx)
                else:
                    last_key = key
            elif tn in ("InstDrain", "InstCall", "InstDMACopy", "InstTriggeredCopy"):
                last_key = None
        for idx in reversed(drop_idx):
            del il[idx]
        n_drop += len(drop_idx)
    return n_drop


def build_nc(B=2, C=512, n_cores=8, repeat=1, phases=("conv", "attn", "proj")):
    """Build the per-core Bass program. B = images per core.

    repeat > 1 emits the whole body multiple times (timing builds only).
    phases: drop "attn"/"proj" for timing-breakdown builds.
    """
    M = C // 128            # channel chunks (co chunks and ci chunks)
    nc = bacc.Bacc("TRN2", target_bir_lowering=False, debug=False,
                   num_devices=n_cores)

    x_d = nc.dram_tensor("x", [B, C, H, H], BF16, kind="ExternalInput")
    wq_d = nc.dram_tensor("wq", [M, 128, 9 * M, 128], BF16, kind="ExternalInput")
    wk_d = nc.dram_tensor("wk", [M, 128, 9 * M, 128], BF16, kind="ExternalInput")
    wv_d = nc.dram_tensor("wv", [M, 128, 9 * M, 128], BF16, kind="ExternalInput")
    pw_d = nc.dram_tensor("pw", [M, 128, C], BF16, kind="ExternalInput")
    bias_d = nc.dram_tensor("biases", [4, C], F32, kind="ExternalInput")
    y_d = nc.dram_tensor("y", [B, C, H, H], F32, kind="ExternalOutput")

    with tile.TileContext(nc) as tc:
        _body(tc, nc, B, M, x_d, (wq_d, wk_d, wv_d), pw_d, bias_d, y_d,
              repeat=repeat, phases=phases)
    nc.compile()
    _strip_redundant_ldws(nc)
    return nc


def _body(tc, nc, B, M, x_d, w_ds, pw_d, bias_d, y_d, repeat=1,
          phases=("conv", "attn", "proj")):
    from contextlib import ExitStack
    ctx = ExitStack()
    C = M * 128
    const = ctx.enter_context(tc.tile_pool(name="const", bufs=1))
    xpad_p = ctx.enter_context(tc.tile_pool(name="xpad", bufs=min(4 * B, 5)))
    w_p = ctx.enter_context(tc.tile_pool(name="wconv", bufs=3))
    qkv_p = ctx.enter_context(tc.tile_pool(name="qkv", bufs=6))
    exp_p = ctx.enter_context(tc.tile_pool(name="exp", bufs=3))
    nt_p = ctx.enter_context(tc.tile_pool(name="normT", bufs=2))
    rc_p = ctx.enter_context(tc.tile_pool(name="recip", bufs=2))
    as_p = ctx.enter_context(tc.tile_pool(name="attns", bufs=2))
    acm_p = ctx.enter_context(tc.tile_pool(name="attncm", bufs=min(M, 4)))
    y_p = ctx.enter_context(tc.tile_pool(name="yout", bufs=4))
    st_p = ctx.enter_context(tc.tile_pool(name="stage", bufs=2))
    cp_ps = ctx.enter_context(tc.tile_pool(name="cpps", bufs=4, space="PSUM"))
    at_ps = ctx.enter_context(tc.tile_pool(name="atps", bufs=2, space="PSUM"))
    nm_ps = ctx.enter_context(tc.tile_pool(name="nmps", bufs=1, space="PSUM"))
    o2_ps = ctx.enter_context(tc.tile_pool(name="o2ps", bufs=1, space="PSUM"))

    # constants
    ones32 = const.tile([128, 32], BF16, tag="ones32")
    nc.gpsimd.memset(ones32[:, :], 1.0)
    # per-partition bias columns: col (ti*M + m) = bias[ti, m*128:(m+1)*128]
    bias_sb = const.tile([128, 4 * M], F32, tag="bias")
    nc.sync.dma_start(out=bias_sb[:, :],
                      in_=bias_d[:, :].rearrange("a (m p) -> p (a m)", p=128))
    pw_sb = const.tile([128, M * C], BF16, tag="pw")
    for k4 in range(M):
        nc.sync.dma_start(out=pw_sb[:, k4 * C:(k4 + 1) * C], in_=pw_d[k4, :, :])

    for b in [b for _ in range(repeat) for b in range(B)]:
        # ---- load x image b as a padded row-major strip [128, 66*66], plus
        # zero tail out to XCOLS so the last psum group's shifted reads land
        # in zeros.
        xpad = []
        for k4 in range(M):
            xp = xpad_p.tile([128, XCOLS], BF16, tag="xpad")
            z = xp[:, :NSTRIP].rearrange("p (r c) -> p r c", c=PW)
            nc.gpsimd.memset(z[:, 0, :], 0.0)
            nc.gpsimd.memset(z[:, PW - 1, :], 0.0)
            nc.gpsimd.memset(z[:, :, 0], 0.0)
            nc.gpsimd.memset(z[:, :, PW - 1], 0.0)
            nc.gpsimd.memset(xp[:, NSTRIP:], 0.0)
            nc.sync.dma_start(out=z[:, 1:H + 1, 1:H + 1],
                              in_=x_d[b, k4 * 128:(k4 + 1) * 128, :, :])
            xpad.append(xp)

        acm = []  # attnout channel-major chunks for proj
        for m in range(M):
            # ---- convs for channel chunk m. Each psum group covers 512
            # consecutive strip positions; rhs slices are fully contiguous.
            # Weight-amortized order: within a group-set, each (k4, t) weight
            # is loaded once and streams GSET matmuls (redundant Ldweights
            # stripped post-compile).
            outs = {}
            for ti, tname in enumerate(("q", "k", "v")):
                w_tile = w_p.tile([128, 9 * M * 128], BF16, tag="wconv")
                nc.sync.dma_start(out=w_tile[:, :],
                                  in_=w_ds[ti][m].rearrange("p a b -> p (a b)"))
                stage = st_p.tile([128, SCOLS], BF16, tag="stage")
                for g0 in range(0, 9, GSET):
                    gs = range(g0, min(g0 + GSET, 9))
                    psums = {}
                    for g in gs:
                        pt = cp_ps.tile([128, 512], F32, tag="cpps")
                        psums[g] = pt
                    for k4 in range(M):
                        for t in range(9):
                            sh = (t // 3) * PW + (t % 3)
                            wsl = w_tile[:, (t * M + k4) * 128:(t * M + k4 + 1) * 128]
                            for g in gs:
                                nc.tensor.matmul(
                                    psums[g][:, :], wsl,
                                    xpad[k4][:, g * 512 + sh: g * 512 + sh + 512],
                                    start=(k4 == 0 and t == 0),
                                    stop=(k4 == M - 1 and t == 8))
                    for g in gs:
                        nc.vector.tensor_scalar_add(
                            stage[:, g * 512:(g + 1) * 512], psums[g][:, :],
                            bias_sb[:, ti * M + m: ti * M + m + 1])
                # stream-transpose row-aligned strided views of the strip into
                # the attention layouts (identical to the old o_t layouts)
                o_t = qkv_p.tile([128, NPOS], BF16, tag="qkv")
                colmajor = (tname == "v")
                zs = stage[:, :NSTRIP].rearrange("p (r c) -> p r c", c=PW)
                for n in range(8):
                    if colmajor:
                        tin = zs[:, 0:H, n * 8:n * 8 + 8].transpose([0, 2, 1])
                    else:
                        tin = zs[:, n * 8:n * 8 + 8, 0:H]
                    tout = o_t[:, n * 512:(n + 1) * 512].rearrange(
                        "p (a c) -> p a c", c=H)
                    nc.vector.transpose(tout, tin)
                outs[tname] = o_t

            if "attn" not in phases:
                nc.gpsimd.dma_start(
                    out=y_d[b, m * 128:(m + 1) * 128].rearrange("p a b -> p (a b)"),
                    in_=outs["q"][:, :])
                continue

            # ---- attention for the 128 channels of chunk m
            o_q, o_k, o_v = outs["q"], outs["k"], outs["v"]
            a_s = as_p.tile([128, NPOS], BF16, tag="attns")
            kks = [o_k[cb * 32:(cb + 1) * 32, :].rearrange(
                "p (kid half c) -> p kid half c", half=2, c=32) for cb in range(4)]
            qqs = [o_q[cb * 32:(cb + 1) * 32, :].rearrange(
                "p (i half c) -> p i half c", half=2, c=32) for cb in range(4)]
            vvs = [o_v[cb * 32:(cb + 1) * 32, :].rearrange(
                "p (w half c) -> p w half c", half=2, c=32) for cb in range(4)]
            for qd in range(8):
                atp = at_ps.tile([128, 512], F32, tag="atps")
                # logits^T:  atp[cb*32+kappa, sl*128+kb*64+i] = sum_j k*q
                # cb innermost so consecutive MMs hit different PE quadrants
                for i1, (sl, kb, jb) in enumerate(
                        (s, k, j) for s in range(4) for k in range(2) for j in range(2)):
                    c = qd * 4 + sl
                    for cb in range(4):
                        nc.tensor.matmul(
                            atp[cb * 32:(cb + 1) * 32,
                                sl * 128 + kb * 64: sl * 128 + (kb + 1) * 64],
                            kks[cb][:, kb * 32:(kb + 1) * 32, jb, c],
                            qqs[cb][:, :, jb, c],
                            start=(i1 == 0), stop=(i1 == 15),
                            skip_group_check=True,
                            tile_position=(cb * 32, cb * 32))
                # exp (fp32 -> bf16), no max subtraction
                ex = exp_p.tile([128, 512], BF16, tag="exp")
                nc.scalar.activation(ex[:, :], atp[:, :], AF.Exp)
                # row sums (over kidx) via ones-matmul, replicated on 32 parts
                nmp = nm_ps.tile([128, 256], F32, tag="nmps")
                for kb in range(2):
                    for cb in range(4):
                        ee = ex[cb * 32:(cb + 1) * 32, :].rearrange(
                            "p (sl half i) -> p sl half i", half=2, i=64)
                        nc.tensor.matmul(
                            nmp[cb * 32:(cb + 1) * 32, :],
                            ones32[cb * 32:(cb + 1) * 32, :],
                            ee[:, :, kb, :],
                            start=(kb == 0), stop=(kb == 1),
                            skip_group_check=True,
                            tile_position=(cb * 32, cb * 32))
                nt = nt_p.tile([128, 256], F32, tag="normT")
                nc.vector.transpose(nt[:, :], nmp[:, :])
                rc = rc_p.tile([128, 8], F32, tag="recip")
                nc.vector.reciprocal(
                    rc[:, :], nt[:, :].rearrange("p (t c) -> p t c", c=32)[:, :, 0])
                # out2 = attn_exp^T' @ v   (unnormalized), K=32 chunks
                o2p = o2_ps.tile([128, 512], F32, tag="o2ps")
                for i2, (sl, ib, kb) in enumerate(
                        (s, i, k) for s in range(4) for i in range(2) for k in range(2)):
                    c = qd * 4 + sl
                    for cb in range(4):
                        nc.tensor.matmul(
                            o2p[cb * 32:(cb + 1) * 32,
                                sl * 128 + ib * 64: sl * 128 + (ib + 1) * 64],
                            ex[cb * 32:(cb + 1) * 32,
                               sl * 128 + kb * 64 + ib * 32:
                               sl * 128 + kb * 64 + ib * 32 + 32],
                            vvs[cb][:, :, kb, c],
                            start=(i2 == 0), stop=(i2 == 15),
                            skip_group_check=True,
                            tile_position=(cb * 32, cb * 32))
                # normalize + write into attnout_s (v-style layout), bf16
                in0 = o2p[:, :].rearrange("p (sl ib w) -> p sl ib w", ib=2, w=64)
                in1 = rc[:, :].rearrange("p (sl ib) -> p sl ib", ib=2)
                in1 = in1.unsqueeze(3).broadcast_to((128, 4, 2, 64))
                outap = a_s[:, :].rearrange("p (t c) -> p t c", c=32)
                outap = outap[:, :, qd * 4:qd * 4 + 4].rearrange(
                    "p (w ib) sl -> p w ib sl", ib=2).transpose([0, 3, 2, 1])
                nc.vector.tensor_tensor(outap, in0, in1, mybir.AluOpType.mult)
            # back-transpose to channel-major (column-major positions)
            a_cm = acm_p.tile([128, NPOS], BF16, tag="attncm")
            nc.vector.transpose(a_cm[:, :], a_s[:, :])
            acm.append(a_cm)

        if "attn" not in phases:
            continue
        if "proj" not in phases:
            for m in range(M):
                nc.gpsimd.dma_start(
                    out=y_d[b, m * 128:(m + 1) * 128].rearrange("p a b -> p (a b)"),
                    in_=acm[m][:, :])
            del acm
            continue

        # ---- proj (1x1 conv with permuted weights) + bias, row-major out
        for mo in range(M):
            for n in range(8):
                yt = y_p.tile([128, NPOS // 8], F32, tag="yout")
                psum = cp_ps.tile([128, 512], F32, tag="cpps")
                for k4 in range(M):
                    rhs = acm[k4][:, :].rearrange("p (w i) -> p w i", i=64)
                    rhs = rhs[:, :, n * 8:(n + 1) * 8].transpose([0, 2, 1])
                    nc.tensor.matmul(
                        psum[:, :],
                        pw_sb[:, k4 * C + mo * 128: k4 * C + (mo + 1) * 128],
                        rhs, start=(k4 == 0), stop=(k4 == M - 1))
                nc.scalar.activation(yt[:, :], psum[:, :], AF.Identity,
                                     bias=bias_sb[:, 3 * M + mo: 3 * M + mo + 1])
                nc.sync.dma_start(
                    out=y_d[b, mo * 128:(mo + 1) * 128, n * 8:(n + 1) * 8, :],
                    in_=yt[:, :])
        del acm
    ctx.close()


def prep_weights(q_w, q_b, kv_w, kv_b, proj_w, proj_b, C=512):
    """Host-side weight re-layouts (numpy, bf16)."""
    M = C // 128
    nh = 16
    cpg = C // nh

    def conv_w(w):
        # w[co, ci, dy, dx] -> [m, p(ci%128), t(=dy*3+dx), k4, co] flat
        w4 = w.reshape(M, 128, M, 128, 3, 3)          # [m, co, k4, p, dy, dx]
        out = np.transpose(w4, (0, 3, 4, 5, 2, 1))    # [m, p, dy, dx, k4, co]
        out = out.reshape(M, 128, 9 * M, 128)
        return np.ascontiguousarray(out).astype(BF)

    wq = conv_w(q_w)
    wk = conv_w(kv_w[:C])
    wv = conv_w(kv_w[C:])
    ch = np.arange(C)
    perm = (ch % cpg) * nh + ch // cpg                # proj input index per attn channel
    pwp = proj_w[:, :, 0, 0][:, perm]                 # [co, ch]
    pw = np.ascontiguousarray(pwp.T.reshape(M, 128, C)).astype(BF)
    biases = np.stack([q_b, kv_b[:C], kv_b[C:], proj_b]).astype(np.float32)
    return wq, wk, wv, pw, biases


_CACHE = {}


def _get_nc():
    if "nc" not in _CACHE:
        _CACHE["nc"] = build_nc(B=2, C=512, n_cores=8)
    return _CACHE["nc"]


def make_in_maps(x, q_w, q_b, kv_w, kv_b, proj_w, proj_b, n_cores=8):
    wq, wk, wv, pw, biases = prep_weights(
        np.asarray(q_w), np.asarray(q_b), np.asarray(kv_w), np.asarray(kv_b),
        np.asarray(proj_w), np.asarray(proj_b))
    x = np.asarray(x, dtype=np.float32).astype(BF)
    bpc = x.shape[0] // n_cores
    return [
        {"x": np.ascontiguousarray(x[i * bpc:(i + 1) * bpc]),
         "wq": wq, "wk": wk, "wv": wv, "pw": pw, "biases": biases}
        for i in range(n_cores)
    ]


def kernel(x, q_w, q_b, kv_w, kv_b, proj_w, proj_b):
    nc = _get_nc()
    in_maps = make_in_maps(x, q_w, q_b, kv_w, kv_b, proj_w, proj_b)
    res = run_bass_kernel_spmd(nc, in_maps, core_ids=list(range(8)))
    out = np.concatenate([res.results[i]["y"] for i in range(8)], axis=0)
    return out.astype(np.float32)



# revision 14
# speedup vs baseline: 640.1495x; 1.1507x over previous
"""Trainium2 Bass kernel for nn_Attention (conv-qkv spatial attention block).

Contract: kernel(**inputs) takes FULL unsharded inputs (B=16, C=512, H=W=64),
shards batch across 8 NeuronCores (2 images per core), runs one SPMD Bass
program, and returns the FULL output (fp32).

Math per image (reference):
  q  = conv3x3(x, q_w) + q_b                      # (C, H, W)
  kv = conv3x3(x, kv_w) + kv_b ; k, v = split(kv)
  per channel ch: attn = softmax(q_ch @ k_ch^T) ; o_ch = attn @ v_ch
  y  = conv1x1(perm(o), proj_w) + proj_b          # head/channel permutation
       (the permutation is folded into proj_w on the host)

Device implementation notes:
  - All matmuls in bf16 with fp32 PSUM accumulation.
  - 3x3 conv = 9 shifted matmuls over a zero-padded bf16 copy of x in SBUF,
    accumulated over ci chunks; bias is added during the ACT evacuation.
  - Per-channel attention operands are produced by DVE stream-transpose
    (32x32 blocks), giving a tiled layout where the spatial index lives on
    partitions mod 32 and attention runs as K=32 matmuls packed 4-wide on
    the PE array via tile_position quadrants.
  - softmax: exp in fp32 without max subtraction (logits bounded ~|75| < 88),
    row sums via a ones-matmul, one reciprocal + broadcast multiply.
"""

import numpy as np
import ml_dtypes

import concourse.bass as bass
import concourse.bacc as bacc
import concourse.mybir as mybir
import concourse.tile as tile
from concourse.bass_utils import run_bass_kernel_spmd

F32 = mybir.dt.float32
BF16 = mybir.dt.bfloat16
AF = mybir.ActivationFunctionType
BF = ml_dtypes.bfloat16

H = 64          # spatial height (attention over rows, contracting cols)
PW = 66         # padded row width
NPOS = H * H    # 4096 positions per image
NSTRIP = PW * PW        # 4356 padded strip positions
XCOLS = 4800            # xpad allocation (group-8 rhs overruns past 4356)
SCOLS = 4608            # 9 psum groups x 512 strip outputs
GSET = 3                # psum groups sharing one weight residency


def _strip_redundant_ldws(nc):
    """Drop InstLdweights identical to the immediately-preceding kept
    Ldweights (same weights AP/tile_position, no waits/updates, no intervening
    PE drain). The following Matmult reuses the already-loaded weights —
    numerically identical either way, but skips the redundant weight-load
    stream on the PE."""
    n_drop = 0
    for bb in nc.m.functions[0].blocks:
        il = bb.instructions
        last_key = None
        drop_idx = []
        for idx, inst in enumerate(il):
            tn = type(inst).__name__
            if tn == "InstLdweights":
                key = inst.concise()
                if key == last_key and not inst.has_wait() and not inst.has_update():
                    drop_idx.append(idx)
                else:
                    last_key = key
            elif tn in ("InstDrain", "InstCall", "InstDMACopy", "InstTriggeredCopy"):
                last_key = None
        for idx in reversed(drop_idx):
            del il[idx]
        n_drop += len(drop_idx)
    return n_drop


def build_nc(B=2, C=512, n_cores=8, repeat=1, phases=("conv", "attn", "proj")):
    """Build the per-core Bass program. B = images per core.

    repeat > 1 emits the whole body multiple times (timing builds only).
    phases: drop "attn"/"proj" for timing-breakdown builds.
    """
    M = C // 128            # channel chunks (co chunks and ci chunks)
    nc = bacc.Bacc("TRN2", target_bir_lowering=False, debug=False,
                   num_devices=n_cores)

    x_d = nc.dram_tensor("x", [B, C, H, H], BF16, kind="ExternalInput")
    wq_d = nc.dram_tensor("wq", [M, 128, 9 * M, 128], BF16, kind="ExternalInput")
    wk_d = nc.dram_tensor("wk", [M, 128, 9 * M, 128], BF16, kind="ExternalInput")
    wv_d = nc.dram_tensor("wv", [M, 128, 9 * M, 128], BF16, kind="ExternalInput")
    pw_d = nc.dram_tensor("pw", [M, 128, C], BF16, kind="ExternalInput")
    bias_d = nc.dram_tensor("biases", [4, C], F32, kind="ExternalInput")
    y_d = nc.dram_tensor("y", [B, C, H, H], F32, kind="ExternalOutput")

    with tile.TileContext(nc) as tc:
        _body(tc, nc, B, M, x_d, (wq_d, wk_d, wv_d), pw_d, bias_d, y_d,
              repeat=repeat, phases=phases)
    nc.compile()
    _strip_redundant_ldws(nc)
    return nc


def _body(tc, nc, B, M, x_d, w_ds, pw_d, bias_d, y_d, repeat=1,
          phases=("conv", "attn", "proj")):
    from contextlib import ExitStack
    ctx = ExitStack()
    C = M * 128
    const = ctx.enter_context(tc.tile_pool(name="const", bufs=1))
    xpad_p = ctx.enter_context(tc.tile_pool(name="xpad", bufs=min(4 * B, 5)))
    w_p = ctx.enter_context(tc.tile_pool(name="wconv", bufs=3))
    qkv_p = ctx.enter_context(tc.tile_pool(name="qkv", bufs=6))
    exp_p = ctx.enter_context(tc.tile_pool(name="exp", bufs=3))
    nt_p = ctx.enter_context(tc.tile_pool(name="normT", bufs=2))
    rc_p = ctx.enter_context(tc.tile_pool(name="recip", bufs=2))
    as_p = ctx.enter_context(tc.tile_pool(name="attns", bufs=2))
    acm_p = ctx.enter_context(tc.tile_pool(name="attncm", bufs=min(M, 4)))
    y_p = ctx.enter_context(tc.tile_pool(name="yout", bufs=4))
    st_p = ctx.enter_context(tc.tile_pool(name="stage", bufs=2))
    cp_ps = ctx.enter_context(tc.tile_pool(name="cpps", bufs=4, space="PSUM"))
    at_ps = ctx.enter_context(tc.tile_pool(name="atps", bufs=2, space="PSUM"))
    nm_ps = ctx.enter_context(tc.tile_pool(name="nmps", bufs=1, space="PSUM"))
    o2_ps = ctx.enter_context(tc.tile_pool(name="o2ps", bufs=1, space="PSUM"))

    # constants
    ones32 = const.tile([128, 32], BF16, tag="ones32")
    nc.gpsimd.memset(ones32[:, :], 1.0)
    # per-partition bias columns: col (ti*M + m) = bias[ti, m*128:(m+1)*128]
    bias_sb = const.tile([128, 4 * M], F32, tag="bias")
    nc.sync.dma_start(out=bias_sb[:, :],
                      in_=bias_d[:, :].rearrange("a (m p) -> p (a m)", p=128))
    pw_sb = const.tile([128, M * C], BF16, tag="pw")
    for k4 in range(M):
        nc.sync.dma_start(out=pw_sb[:, k4 * C:(k4 + 1) * C], in_=pw_d[k4, :, :])

    def load_x(b):
        # load x image b as a padded row-major strip [128, 66*66], plus zero
        # tail out to XCOLS so the last psum group's shifted reads land in
        # zeros.
        xpad = []
        for k4 in range(M):
            xp = xpad_p.tile([128, XCOLS], BF16, tag="xpad")
            z = xp[:, :NSTRIP].rearrange("p (r c) -> p r c", c=PW)
            nc.gpsimd.memset(z[:, 0, :], 0.0)
            nc.gpsimd.memset(z[:, PW - 1, :], 0.0)
            nc.gpsimd.memset(z[:, :, 0], 0.0)
            nc.gpsimd.memset(z[:, :, PW - 1], 0.0)
            nc.gpsimd.memset(xp[:, NSTRIP:], 0.0)
            nc.sync.dma_start(out=z[:, 1:H + 1, 1:H + 1],
                              in_=x_d[b, k4 * 128:(k4 + 1) * 128, :, :])
            xpad.append(xp)
        return xpad

    # group 8 only needs strip positions 4096..4222 -> N=128
    gn = {g: (128 if g == 8 else 512) for g in range(9)}

    def conv_chunk(xpad, m):
        # Each psum group covers 512 consecutive strip positions; rhs slices
        # are fully contiguous. Weight-amortized order: within a group-set,
        # each (k4, t) weight is loaded once and streams GSET matmuls
        # (redundant Ldweights stripped post-compile).
        outs = {}
        for ti, tname in enumerate(("q", "k", "v")):
            w_tile = w_p.tile([128, 9 * M * 128], BF16, tag="wconv")
            nc.sync.dma_start(out=w_tile[:, :],
                              in_=w_ds[ti][m].rearrange("p a b -> p (a b)"))
            stage = st_p.tile([128, SCOLS], BF16, tag="stage")
            for g0 in range(0, 9, GSET):
                gs = range(g0, min(g0 + GSET, 9))
                psums = {}
                for g in gs:
                    pt = cp_ps.tile([128, 512], F32, tag="cpps")
                    psums[g] = pt
                for k4 in range(M):
                    for t in range(9):
                        sh = (t // 3) * PW + (t % 3)
                        wsl = w_tile[:, (t * M + k4) * 128:(t * M + k4 + 1) * 128]
                        for g in gs:
                            nc.tensor.matmul(
                                psums[g][:, :gn[g]], wsl,
                                xpad[k4][:, g * 512 + sh: g * 512 + sh + gn[g]],
                                start=(k4 == 0 and t == 0),
                                stop=(k4 == M - 1 and t == 8))
                for g in gs:
                    nc.scalar.activation(
                        stage[:, g * 512:g * 512 + gn[g]], psums[g][:, :gn[g]],
                        AF.Identity,
                        bias=bias_sb[:, ti * M + m: ti * M + m + 1])
            # stream-transpose row-aligned strided views of the strip into
            # the attention layouts (identical to the old o_t layouts)
            o_t = qkv_p.tile([128, NPOS], BF16, tag="qkv")
            colmajor = (tname == "v")
            zs = stage[:, :NSTRIP].rearrange("p (r c) -> p r c", c=PW)
            for n in range(8):
                if colmajor:
                    tin = zs[:, 0:H, n * 8:n * 8 + 8].transpose([0, 2, 1])
                else:
                    tin = zs[:, n * 8:n * 8 + 8, 0:H]
                tout = o_t[:, n * 512:(n + 1) * 512].rearrange(
                    "p (a c) -> p a c", c=H)
                nc.vector.transpose(tout, tin)
            outs[tname] = o_t
        return outs

    def attn_chunk(outs):
        # attention for the 128 channels of one chunk
        o_q, o_k, o_v = outs["q"], outs["k"], outs["v"]
        a_s = as_p.tile([128, NPOS], BF16, tag="attns")
        kks = [o_k[cb * 32:(cb + 1) * 32, :].rearrange(
            "p (kid half c) -> p kid half c", half=2, c=32) for cb in range(4)]
        qqs = [o_q[cb * 32:(cb + 1) * 32, :].rearrange(
            "p (i half c) -> p i half c", half=2, c=32) for cb in range(4)]
        vvs = [o_v[cb * 32:(cb + 1) * 32, :].rearrange(
            "p (w half c) -> p w half c", half=2, c=32) for cb in range(4)]
        for qd in range(8):
            atp = at_ps.tile([128, 512], F32, tag="atps")
            # logits^T:  atp[cb*32+kappa, sl*128+kb*64+i] = sum_j k*q
            # cb innermost so consecutive MMs hit different PE quadrants
            for i1, (sl, kb, jb) in enumerate(
                    (s, k, j) for s in range(4) for k in range(2) for j in range(2)):
                c = qd * 4 + sl
                for cb in range(4):
                    nc.tensor.matmul(
                        atp[cb * 32:(cb + 1) * 32,
                            sl * 128 + kb * 64: sl * 128 + (kb + 1) * 64],
                        kks[cb][:, kb * 32:(kb + 1) * 32, jb, c],
                        qqs[cb][:, :, jb, c],
                        start=(i1 == 0), stop=(i1 == 15),
                        skip_group_check=True,
                        tile_position=(cb * 32, cb * 32))
            # exp (fp32 -> bf16), no max subtraction
            ex = exp_p.tile([128, 512], BF16, tag="exp")
            nc.scalar.activation(ex[:, :], atp[:, :], AF.Exp)
            # row sums (over kidx) via ones-matmul, replicated on 32 parts
            nmp = nm_ps.tile([128, 256], F32, tag="nmps")
            for cb in range(4):
                ee = ex[cb * 32:(cb + 1) * 32, :].rearrange(
                    "p (sl half i) -> p sl half i", half=2, i=64)
                for kb in range(2):
                    nc.tensor.matmul(
                        nmp[cb * 32:(cb + 1) * 32, :],
                        ones32[cb * 32:(cb + 1) * 32, :],
                        ee[:, :, kb, :],
                        start=(kb == 0), stop=(kb == 1),
                        skip_group_check=True,
                        tile_position=(cb * 32, cb * 32))
            nt = nt_p.tile([128, 256], F32, tag="normT")
            nc.vector.transpose(nt[:, :], nmp[:, :])
            rc = rc_p.tile([128, 8], F32, tag="recip")
            nc.vector.reciprocal(
                rc[:, :], nt[:, :].rearrange("p (t c) -> p t c", c=32)[:, :, 0])
            # out2 = attn_exp^T' @ v   (unnormalized), K=32 chunks
            o2p = o2_ps.tile([128, 512], F32, tag="o2ps")
            for i2, (sl, ib, kb) in enumerate(
                    (s, i, k) for s in range(4) for i in range(2) for k in range(2)):
                c = qd * 4 + sl
                for cb in range(4):
                    nc.tensor.matmul(
                        o2p[cb * 32:(cb + 1) * 32,
                            sl * 128 + ib * 64: sl * 128 + (ib + 1) * 64],
                        ex[cb * 32:(cb + 1) * 32,
                           sl * 128 + kb * 64 + ib * 32:
                           sl * 128 + kb * 64 + ib * 32 + 32],
                        vvs[cb][:, :, kb, c],
                        start=(i2 == 0), stop=(i2 == 15),
                        skip_group_check=True,
                        tile_position=(cb * 32, cb * 32))
            # normalize + write into attnout_s (v-style layout), bf16
            in0 = o2p[:, :].rearrange("p (sl ib w) -> p sl ib w", ib=2, w=64)
            in1 = rc[:, :].rearrange("p (sl ib) -> p sl ib", ib=2)
            in1 = in1.unsqueeze(3).broadcast_to((128, 4, 2, 64))
            outap = a_s[:, :].rearrange("p (t c) -> p t c", c=32)
            outap = outap[:, :, qd * 4:qd * 4 + 4].rearrange(
                "p (w ib) sl -> p w ib sl", ib=2).transpose([0, 3, 2, 1])
            nc.vector.tensor_tensor(outap, in0, in1, mybir.AluOpType.mult)
        # back-transpose to channel-major, stored ROW-major (col = i*64 + w)
        # so the proj rhs slices are contiguous
        a_cm = acm_p.tile([128, NPOS], BF16, tag="attncm")
        tout = a_cm[:, :].rearrange("p (i w) -> p w i", w=64)
        nc.vector.transpose(tout,
                            a_s[:, :].rearrange("p (w i) -> p w i", i=64))
        return a_cm

    def proj_image(b, acm):
        # proj (1x1 conv with permuted weights) + bias, row-major out
        for mo in range(M):
            for n in range(8):
                yt = y_p.tile([128, NPOS // 8], F32, tag="yout")
                psum = cp_ps.tile([128, 512], F32, tag="cpps")
                for k4 in range(M):
                    nc.tensor.matmul(
                        psum[:, :],
                        pw_sb[:, k4 * C + mo * 128: k4 * C + (mo + 1) * 128],
                        acm[k4][:, n * 512:(n + 1) * 512],
                        start=(k4 == 0), stop=(k4 == M - 1))
                nc.scalar.activation(yt[:, :], psum[:, :], AF.Identity,
                                     bias=bias_sb[:, 3 * M + mo: 3 * M + mo + 1])
                nc.sync.dma_start(
                    out=y_d[b, mo * 128:(mo + 1) * 128, n * 8:(n + 1) * 8, :],
                    in_=yt[:, :])

    images = [b for _ in range(repeat) for b in range(B)]

    if "attn" not in phases:
        for b in images:
            xpad = load_x(b)
            for m in range(M):
                outs = conv_chunk(xpad, m)
                nc.gpsimd.dma_start(
                    out=y_d[b, m * 128:(m + 1) * 128].rearrange("p a b -> p (a b)"),
                    in_=outs["q"][:, :])
        ctx.close()
        return

    def retire(pend, acm_of):
        s, pm, pouts = pend
        acm_of.setdefault(s, []).append(attn_chunk(pouts))
        if pm == M - 1:
            b = images[s]
            if "proj" in phases:
                proj_image(b, acm_of.pop(s))
            else:
                for mm, a in enumerate(acm_of.pop(s)):
                    nc.gpsimd.dma_start(
                        out=y_d[b, mm * 128:(mm + 1) * 128].rearrange(
                            "p a b -> p (a b)"),
                        in_=a[:, :])

    # software pipeline: attention/proj for a chunk are emitted under the
    # NEXT chunk's conv matmuls so the PE never waits on the evac chains
    pend = None
    acm_of = {}
    for s, b in enumerate(images):
        xpad = load_x(b)
        for m in range(M):
            outs = conv_chunk(xpad, m)
            if pend is not None:
                retire(pend, acm_of)
            pend = (s, m, outs)
    if pend is not None:
        retire(pend, acm_of)
    ctx.close()


def prep_weights(q_w, q_b, kv_w, kv_b, proj_w, proj_b, C=512):
    """Host-side weight re-layouts (numpy, bf16)."""
    M = C // 128
    nh = 16
    cpg = C // nh

    def conv_w(w):
        # w[co, ci, dy, dx] -> [m, p(ci%128), t(=dy*3+dx), k4, co] flat
        w4 = w.reshape(M, 128, M, 128, 3, 3)          # [m, co, k4, p, dy, dx]
        out = np.transpose(w4, (0, 3, 4, 5, 2, 1))    # [m, p, dy, dx, k4, co]
        out = out.reshape(M, 128, 9 * M, 128)
        return np.ascontiguousarray(out).astype(BF)

    wq = conv_w(q_w)
    wk = conv_w(kv_w[:C])
    wv = conv_w(kv_w[C:])
    ch = np.arange(C)
    perm = (ch % cpg) * nh + ch // cpg                # proj input index per attn channel
    pwp = proj_w[:, :, 0, 0][:, perm]                 # [co, ch]
    pw = np.ascontiguousarray(pwp.T.reshape(M, 128, C)).astype(BF)
    biases = np.stack([q_b, kv_b[:C], kv_b[C:], proj_b]).astype(np.float32)
    return wq, wk, wv, pw, biases


_CACHE = {}


def _get_nc():
    if "nc" not in _CACHE:
        _CACHE["nc"] = build_nc(B=2, C=512, n_cores=8)
    return _CACHE["nc"]


def make_in_maps(x, q_w, q_b, kv_w, kv_b, proj_w, proj_b, n_cores=8):
    wq, wk, wv, pw, biases = prep_weights(
        np.asarray(q_w), np.asarray(q_b), np.asarray(kv_w), np.asarray(kv_b),
        np.asarray(proj_w), np.asarray(proj_b))
    x = np.asarray(x, dtype=np.float32).astype(BF)
    bpc = x.shape[0] // n_cores
    return [
        {"x": np.ascontiguousarray(x[i * bpc:(i + 1) * bpc]),
         "wq": wq, "wk": wk, "wv": wv, "pw": pw, "biases": biases}
        for i in range(n_cores)
    ]


def kernel(x, q_w, q_b, kv_w, kv_b, proj_w, proj_b):
    nc = _get_nc()
    in_maps = make_in_maps(x, q_w, q_b, kv_w, kv_b, proj_w, proj_b)
    res = run_bass_kernel_spmd(nc, in_maps, core_ids=list(range(8)))
    out = np.concatenate([res.results[i]["y"] for i in range(8)], axis=0)
    return out.astype(np.float32)

